# revision 10
# baseline (speedup 1.0000x reference)
"""MoRALinear fused kernel for 8x TRN2 NeuronCores — bf16 v4.

Math: reference computes
    y = x @ W.T + b + tile(lora_A(chunk_sum(x)))
Both the chunk-sum (x -> [B,S,r] by summing 4 chunks of 1024) and the
output tiling (repeat r->4096) are linear maps made of stacked identity
blocks, so the adapter folds into the base weight exactly:
    y = x @ (W + tile(A, (4,4))).T + b
The device kernel is then a single dense GEMM [16384,4096]x[4096,4096]
plus a bias, data-parallel over tokens across 8 cores (weights
replicated; no collectives).

Design (per core: M=2048 tokens, K=4096, N=4096):
  - Operands are bf16: the PE runs bf16 at the same 1 elem/cycle as
    float32r, but LDWEIGHTS gets FWL (hides fully under the matmul
    stream: measured 216 ns/MM = the N=512 issue floor) and HBM traffic
    halves. Accumulation stays fp32 in PSUM; rel err ~4e-3.
  - Loop order: m-group (2 x 1024 tokens) outer, n-tile (8 x 512) inner.
    W streams twice (2 x 32 MB, trivially within bandwidth); each
    m-group's x chunks are SBUF-resident across its 8 n-tiles. This
    leaves every DMA far off the critical path: the only dense window is
    the first (mg0, nt0) tile, which streams just its own x+W.
  - 8 PSUM banks hold the 8 m-strips of one (mg, nt) group; bias is
    added during PSUM->SBUF eviction on the vector engine, emitting bf16
    (2x DVE rate, half the out-DMA bytes; host upcasts). The last
    k-chunk runs m-outer so the 8 evictions spread over ~7 us instead of
    bunching into a serial tail chain.
  - ~20 warm-up matmuls on a zeroed SBUF tile run during the initial
    DMA wait so the PE's HAM clock gate reaches 8/8 (2.4 GHz) before the
    first real matmul, and the first data tile streams per-k-strip with
    x leading W so the PE can start ~4 us after the DMA queue opens.
"""

import numpy as np
import ml_dtypes

import concourse.bass as bass
import concourse.mybir as mybir
import concourse.tile as tile
from concourse import bacc
from concourse.bass_utils import run_bass_kernel_spmd

B, S, IN_F, OUT_F = 4, 4096, 4096, 4096
N_CORES = 8
TOKENS = B * S                  # 16384
M_PER_CORE = TOKENS // N_CORES  # 2048

P = 128
KO = IN_F // P                  # 32 k-strips
NTILE = 512
NT = OUT_F // NTILE             # 8 n-tiles
KO_CHUNK = 4                    # k-strips per chunk
MB = 1024                       # tokens per m-group (8 PSUM banks x 128)
N_WARM = 14                     # HAM warm-up matmuls

F32 = mybir.dt.float32
BF16 = mybir.dt.bfloat16
NP_BF16 = ml_dtypes.bfloat16


def build_nc(m_per_core: int = M_PER_CORE, mb: int = MB):
    assert m_per_core % mb == 0
    m_groups = m_per_core // mb          # 2
    n_chunks = KO // KO_CHUNK            # 8
    mt = mb // P                         # 8 m-strips per group
    nc = bacc.Bacc("TRN2", target_bir_lowering=False, debug=False)

    with tile.TileContext(nc) as tc:
        with tc.tile_pool(name="dram", bufs=1, space="DRAM") as dram:
            # x swizzled group-major: row mg*KO+ko holds k-strip ko of token
            # group mg, so each chunk load is one contiguous run/partition
            xt = dram.tile(
                [P, m_groups * KO, mb], BF16, kind="ExternalInput", name="xt",
                uniquify=False,
            )
            # W swizzled n-tile-major: row nt*KO+ko holds W_eff.T[k-strip ko,
            # n-slice nt] so each chunk load is one contiguous run/partition
            wt = dram.tile(
                [P, NT * KO, NTILE], BF16, kind="ExternalInput", name="wt",
                uniquify=False,
            )
            bias_in = dram.tile(
                [P, OUT_F], F32, kind="ExternalInput", name="bias", uniquify=False
            )
            out_d = dram.tile(
                [P, m_per_core // P, OUT_F], BF16, kind="ExternalOutput", name="out",
                uniquify=False,
            )

        with (
            tc.tile_pool(name="const", bufs=1) as const,
            tc.tile_pool(name="xpool", bufs=m_groups * n_chunks) as xpool,
            tc.tile_pool(name="wpool", bufs=n_chunks + 2) as wpool,
            tc.tile_pool(name="opool", bufs=8) as opool,
            tc.tile_pool(name="pspool", bufs=8, space="PSUM") as pspool,
        ):
            bias_sb = const.tile([P, OUT_F], F32, name="bias_sb")

            # HAM warm-up: keep the PE busy while the first x/W chunks
            # stream in, so the clock gate is at 8/8 when real work starts
            # and the activity window never sees a >3.4us idle gap.
            warm_w = const.tile([P, P], BF16, name="warm_w")
            warm_x = const.tile([P, NTILE], BF16, name="warm_x")
            nc.gpsimd.memset(warm_w[:], 0.0)
            nc.gpsimd.memset(warm_x[:], 0.0)

            # x chunks stay resident for their whole m-group
            xchunks = [[None] * n_chunks for _ in range(m_groups)]
            wk_cur = [None] * n_chunks   # W chunks of the current n-tile
            wk_next = [None] * n_chunks  # prefetched chunks of the next tile

            def load_xchunk(mg, ko4):
                xc = xpool.tile([P, KO_CHUNK, mb], BF16, name="xc")
                row = mg * KO + ko4 * KO_CHUNK
                nc.sync.dma_start(xc[:], xt[:, row : row + KO_CHUNK, :])
                return xc

            def load_cold_chunks(ko4, halve_x):
                """Cold-start loads for the first tile: x and W interleaved
                per k-strip so the ring arrival order matches the kj-inner
                consumption order; the first k-strip's x is split into token
                halves so the first m-strips' matmuls start even earlier."""
                xc = xpool.tile([P, KO_CHUNK, mb], BF16, name="xc")
                wk = wpool.tile([P, KO_CHUNK, NTILE], BF16, name="wk")
                for kj in range(KO_CHUNK):
                    row = ko4 * KO_CHUNK + kj
                    if halve_x and kj == 0:
                        nc.sync.dma_start(
                            xc[:, 0:1, : mb // 2], xt[:, row : row + 1, : mb // 2]
                        )
                        nc.sync.dma_start(
                            wk[:, 0:1, :], wt[:, row : row + 1, :]
                        )
                        nc.sync.dma_start(
                            xc[:, 0:1, mb // 2 :], xt[:, row : row + 1, mb // 2 :]
                        )
                    else:
                        nc.sync.dma_start(
                            xc[:, kj : kj + 1, :], xt[:, row : row + 1, :]
                        )
                        nc.sync.dma_start(
                            wk[:, kj : kj + 1, :], wt[:, row : row + 1, :]
                        )
                return xc, wk

            def load_wchunk(nt, ko4):
                wk = wpool.tile([P, KO_CHUNK, NTILE], BF16, name="wk")
                row = nt * KO + ko4 * KO_CHUNK
                nc.sync.dma_start(wk[:], wt[:, row : row + KO_CHUNK, :])
                return wk

            def do_mm(psums, xc, wk, kj, m, ko4, nt, mg):
                ko = ko4 * KO_CHUNK + kj
                nc.tensor.matmul(
                    psums[m][:],
                    lhsT=xc[:, kj : kj + 1, m * P : (m + 1) * P],
                    rhs=wk[:, kj : kj + 1, :],
                    start=(ko == 0),
                    stop=(ko == KO - 1),
                )
                if ko == KO - 1:
                    ns = slice(nt * NTILE, (nt + 1) * NTILE)
                    ot = opool.tile([P, NTILE], BF16, name="ot")
                    nc.vector.tensor_add(
                        out=ot[:], in0=psums[m][:], in1=bias_sb[:, ns]
                    )
                    nc.sync.dma_start(out_d[:, mg * mt + m, ns], ot[:])

            seq = [(mg, nt) for mg in range(m_groups) for nt in range(NT)]
            for si, (mg, nt) in enumerate(seq):
                psums = [
                    pspool.tile([P, NTILE], F32, name="ps") for _ in range(mt)
                ]
                if si == 0:
                    # HAM warm-up into psums[0]: the first real matmul's
                    # start=True clears the bank, so the zeros never escape
                    for _ in range(N_WARM):
                        nc.tensor.matmul(
                            psums[0][:], lhsT=warm_w[:], rhs=warm_x[:],
                            start=True, stop=True,
                        )
                for ko4 in range(n_chunks):
                    if si == 0:
                        xchunks[0][ko4], wk_cur[ko4] = load_cold_chunks(
                            ko4, halve_x=(ko4 == 0)
                        )
                    wk = wk_cur[ko4]
                    xc = xchunks[mg][ko4]
                    if ko4 == n_chunks - 1:
                        # m-outer: each strip finishes its k-accumulation
                        # 4 MMs apart, spreading evictions across ~7us
                        for m in range(mt):
                            for kj in range(KO_CHUNK):
                                do_mm(psums, xc, wk, kj, m, ko4, nt, mg)
                    else:
                        for kj in range(KO_CHUNK):
                            for m in range(mt):
                                do_mm(psums, xc, wk, kj, m, ko4, nt, mg)
                    # trailing loads, emitted after this step's matmuls so
                    # they queue behind the critical stream:
                    #  - first tile: bias quarters + next-tile W ride the
                    #    back half of the cold start
                    #  - mg0/nt2..3: mg1's x chunks trickle in (needed at
                    #    t ~ 450us, loaded by ~250us)
                    #  - otherwise: one W chunk of the next tile per step
                    if si == 0:
                        if ko4 >= 4:
                            q = ko4 - 4
                            nc.sync.dma_start(
                                bias_sb[:, q * 1024 : (q + 1) * 1024],
                                bias_in[:, q * 1024 : (q + 1) * 1024],
                            )
                            wk_next[2 * q] = load_wchunk(seq[1][1], 2 * q)
                            wk_next[2 * q + 1] = load_wchunk(seq[1][1], 2 * q + 1)
                    else:
                        if mg == 0 and nt in (2, 3) and ko4 % 2 == 1:
                            c = (nt - 2) * 4 + ko4 // 2
                            xchunks[1][c] = load_xchunk(1, c)
                        if si + 1 < len(seq):
                            wk_next[ko4] = load_wchunk(seq[si + 1][1], ko4)
                if si + 1 < len(seq):
                    for c in range(n_chunks):
                        wk_cur[c] = wk_next[c]

    nc.compile()
    return nc


def prep_inputs(x, W, b, A, m_per_core: int = M_PER_CORE, n_cores: int = N_CORES):
    """Host-side shard + layout prep. Returns in_maps for run_bass_kernel_spmd."""
    x = np.asarray(x, dtype=np.float32)
    W = np.asarray(W, dtype=np.float32)
    b = np.asarray(b, dtype=np.float32)
    A = np.asarray(A, dtype=np.float32)

    r = A.shape[0]
    w_eff = W + np.tile(A, (OUT_F // r, IN_F // r))
    # wt[p, nt*KO + ko, j] = w_eff[nt*512 + j, ko*128 + p]
    wt = np.ascontiguousarray(
        w_eff.reshape(NT, NTILE, KO, P)
        .transpose(3, 0, 2, 1)
        .reshape(P, NT * KO, NTILE)
        .astype(NP_BF16)
    )
    bias = np.ascontiguousarray(np.broadcast_to(b, (P, OUT_F)))

    x_flat = x.reshape(TOKENS, IN_F)
    m_groups = m_per_core // MB
    in_maps = []
    for c in range(n_cores):
        shard = x_flat[c * m_per_core : (c + 1) * m_per_core]
        # xt[p, mg*KO + ko, m] = shard[mg*MB + m, ko*128 + p]
        xt = np.ascontiguousarray(
            shard.reshape(m_groups, MB, KO, P)
            .transpose(3, 0, 2, 1)
            .reshape(P, m_groups * KO, MB)
            .astype(NP_BF16)
        )
        in_maps.append({"xt": xt, "wt": wt, "bias": bias})
    return in_maps


def unshard(results, m_per_core: int = M_PER_CORE):
    shards = []
    for res in results:
        o = np.asarray(res["out"], dtype=np.float32)
        # [P, m_per_core//P, OUT_F]; token = strip*128 + p
        shards.append(o.transpose(1, 0, 2).reshape(m_per_core, OUT_F))
    return np.concatenate(shards, axis=0).reshape(B, S, OUT_F)


_NC_CACHE = {}


def run(x, W, b, A, trace=False, tmpdir=None, **spmd_kwargs):
    key = (M_PER_CORE, MB)
    if key not in _NC_CACHE:
        _NC_CACHE[key] = build_nc()
    nc = _NC_CACHE[key]
    in_maps = prep_inputs(x, W, b, A)
    br = run_bass_kernel_spmd(
        nc, in_maps, list(range(N_CORES)), trace=trace, tmpdir=tmpdir, **spmd_kwargs
    )
    return unshard(br.results), br


def kernel(x, W, b, A):
    last_err = None
    for attempt in range(3):
        try:
            out, _ = run(x, W, b, A)
            return out.astype(np.float32)
        except Exception as e:  # transient device flakes (e.g. NRT exec errors)
            last_err = e
            _NC_CACHE.clear()
            import time

            time.sleep(5)
    raise last_err


# revision 12
# speedup vs baseline: 1.1989x; 1.1989x over previous
"""MoRALinear fused kernel for 8x TRN2 NeuronCores — bf16 v5.

Math: reference computes
    y = x @ W.T + b + tile(lora_A(chunk_sum(x)))
Both the chunk-sum (x -> [B,S,r] by summing 4 chunks of 1024) and the
output tiling (repeat r->4096) are linear maps made of stacked identity
blocks, so the adapter folds into the base weight exactly:
    y = x @ (W + tile(A, (4,4))).T + b
The device kernel is then a single dense GEMM [16384,4096]x[4096,4096]
plus a bias, data-parallel over tokens across 8 cores (weights
replicated; no collectives).

Design (per core: M=2048 tokens, K=4096, N=4096):
  - Operands are bf16: the PE runs bf16 at the same 1 elem/cycle as
    float32r, but LDWEIGHTS gets FWL (hides fully under the matmul
    stream: measured 216 ns/MM = the N=512 issue floor) and HBM traffic
    halves. Accumulation stays fp32 in PSUM; rel err ~4e-3.
  - Loop order: n-tile outer, m-group inner. The whole 2048-token x
    shard is SBUF-resident (16 chunks, 128 KiB/part) and W streams
    exactly once (8 chunks per n-tile, reused by both m-groups) —
    minimum HBM traffic also keeps the package out of the sustained-
    power P0 downclock (a W-per-tile variant measured 2.0 GHz all run).
  - 8 PSUM banks hold the 8 m-strips of one (n-tile, m-group); bias is
    added during PSUM->SBUF eviction on the vector engine, emitting bf16
    (2x DVE rate, half the out-DMA bytes; host upcasts). The last
    k-chunk runs m-outer so the 8 evictions spread over ~7 us instead of
    bunching into a serial tail chain.
  - ~14 warm-up matmuls on a zeroed SBUF tile run during the initial
    DMA wait so the PE's HAM clock gate reaches 8/8 (2.4 GHz) before
    the first real matmul; the first tile streams per-k-strip with x
    leading W, and bias rides in quarters behind the cold-start stream.
"""

import numpy as np
import ml_dtypes

import concourse.bass as bass
import concourse.mybir as mybir
import concourse.tile as tile
from concourse import bacc
from concourse.bass_utils import run_bass_kernel_spmd

B, S, IN_F, OUT_F = 4, 4096, 4096, 4096
N_CORES = 8
TOKENS = B * S                  # 16384
M_PER_CORE = TOKENS // N_CORES  # 2048

P = 128
KO = IN_F // P                  # 32 k-strips
NTILE = 512
NT = OUT_F // NTILE             # 8 n-tiles
KO_CHUNK = 4                    # k-strips per chunk
MB = 1024                       # tokens per m-group (8 PSUM banks x 128)
N_WARM = 14                     # HAM warm-up matmuls

F32 = mybir.dt.float32
BF16 = mybir.dt.bfloat16
NP_BF16 = ml_dtypes.bfloat16


def build_nc(m_per_core: int = M_PER_CORE, mb: int = MB):
    assert m_per_core % mb == 0
    m_groups = m_per_core // mb          # 2
    n_chunks = KO // KO_CHUNK            # 8
    mt = mb // P                         # 8 m-strips per group
    nc = bacc.Bacc("TRN2", target_bir_lowering=False, debug=False)

    with tile.TileContext(nc) as tc:
        with tc.tile_pool(name="dram", bufs=1, space="DRAM") as dram:
            # x swizzled group-major: row mg*KO+ko holds k-strip ko of token
            # group mg, so each chunk load is one contiguous run/partition
            xt = dram.tile(
                [P, m_groups * KO, mb], BF16, kind="ExternalInput", name="xt",
                uniquify=False,
            )
            # W swizzled n-tile-major: row nt*KO+ko holds W_eff.T[k-strip ko,
            # n-slice nt] so each chunk load is one contiguous run/partition
            wt = dram.tile(
                [P, NT * KO, NTILE], BF16, kind="ExternalInput", name="wt",
                uniquify=False,
            )
            bias_in = dram.tile(
                [P, OUT_F], F32, kind="ExternalInput", name="bias", uniquify=False
            )
            out_d = dram.tile(
                [P, m_per_core // P, OUT_F], BF16, kind="ExternalOutput", name="out",
                uniquify=False,
            )

        with (
            tc.tile_pool(name="const", bufs=1) as const,
            tc.tile_pool(name="xpool", bufs=m_groups * n_chunks) as xpool,
            tc.tile_pool(name="wpool", bufs=n_chunks + 2) as wpool,
            tc.tile_pool(name="opool", bufs=8) as opool,
            tc.tile_pool(name="pspool", bufs=8, space="PSUM") as pspool,
        ):
            bias_sb = const.tile([P, OUT_F], F32, name="bias_sb")

            # HAM warm-up operands: zeroed by GpSimd (otherwise idle) so
            # the warm-up matmuls depend on nothing but the preamble
            warm_w = const.tile([P, P], BF16, name="warm_w")
            warm_x = const.tile([P, NTILE], BF16, name="warm_x")
            nc.gpsimd.memset(warm_w[:], 0.0)
            nc.gpsimd.memset(warm_x[:], 0.0)

            # x chunks stay resident for the whole kernel
            xchunks = [[None] * n_chunks for _ in range(m_groups)]
            wk_cur = [None] * n_chunks   # W chunks of the current n-tile
            wk_next = [None] * n_chunks  # prefetched chunks of the next n-tile

            def load_xchunk(mg, ko4):
                xc = xpool.tile([P, KO_CHUNK, mb], BF16, name="xc")
                row = mg * KO + ko4 * KO_CHUNK
                nc.sync.dma_start(xc[:], xt[:, row : row + KO_CHUNK, :])
                return xc

            def load_cold_chunks(ko4, halve_x):
                """Cold-start loads for (nt0, mg0): x and W interleaved per
                k-strip (x leading) so the ring arrival order matches the
                kj-inner consumption order; the first k-strip's x is split
                into token halves so the first matmuls start even earlier."""
                xc = xpool.tile([P, KO_CHUNK, mb], BF16, name="xc")
                wk = wpool.tile([P, KO_CHUNK, NTILE], BF16, name="wk")
                for kj in range(KO_CHUNK):
                    row = ko4 * KO_CHUNK + kj
                    if halve_x and kj == 0:
                        nc.sync.dma_start(
                            xc[:, 0:1, : mb // 2], xt[:, row : row + 1, : mb // 2]
                        )
                        nc.sync.dma_start(wk[:, 0:1, :], wt[:, row : row + 1, :])
                        nc.sync.dma_start(
                            xc[:, 0:1, mb // 2 :], xt[:, row : row + 1, mb // 2 :]
                        )
                    else:
                        nc.sync.dma_start(
                            xc[:, kj : kj + 1, :], xt[:, row : row + 1, :]
                        )
                        nc.sync.dma_start(
                            wk[:, kj : kj + 1, :], wt[:, row : row + 1, :]
                        )
                return xc, wk

            def load_wchunk(nt, ko4):
                wk = wpool.tile([P, KO_CHUNK, NTILE], BF16, name="wk")
                row = nt * KO + ko4 * KO_CHUNK
                nc.sync.dma_start(wk[:], wt[:, row : row + KO_CHUNK, :])
                return wk

            def do_mm(psums, xc, wk, kj, m, ko4, nt, mg):
                ko = ko4 * KO_CHUNK + kj
                nc.tensor.matmul(
                    psums[m][:],
                    lhsT=xc[:, kj : kj + 1, m * P : (m + 1) * P],
                    rhs=wk[:, kj : kj + 1, :],
                    start=(ko == 0),
                    stop=(ko == KO - 1),
                )
                if ko == KO - 1:
                    ns = slice(nt * NTILE, (nt + 1) * NTILE)
                    ot = opool.tile([P, NTILE], BF16, name="ot")
                    nc.vector.tensor_add(
                        out=ot[:], in0=psums[m][:], in1=bias_sb[:, ns]
                    )
                    nc.sync.dma_start(out_d[:, mg * mt + m, ns], ot[:])

            for nt in range(NT):
                for mg in range(m_groups):
                    psums = [
                        pspool.tile([P, NTILE], F32, name="ps") for _ in range(mt)
                    ]
                    if nt == 0 and mg == 0:
                        # HAM warm-up into psums[0]: the first real matmul's
                        # start=True clears the bank, so nothing escapes
                        for _ in range(N_WARM):
                            nc.tensor.matmul(
                                psums[0][:], lhsT=warm_w[:], rhs=warm_x[:],
                                start=True, stop=True,
                            )
                    for ko4 in range(n_chunks):
                        if nt == 0 and mg == 0:
                            xchunks[0][ko4], wk_cur[ko4] = load_cold_chunks(
                                ko4, halve_x=(ko4 == 0)
                            )
                        wk = wk_cur[ko4]
                        xc = xchunks[mg][ko4]
                        if ko4 == n_chunks - 1:
                            # m-outer: each strip finishes its k-accumulation
                            # 4 MMs apart, spreading evictions across ~7us
                            for m in range(mt):
                                for kj in range(KO_CHUNK):
                                    do_mm(psums, xc, wk, kj, m, ko4, nt, mg)
                        else:
                            for kj in range(KO_CHUNK):
                                for m in range(mt):
                                    do_mm(psums, xc, wk, kj, m, ko4, nt, mg)
                        # trailing loads, emitted after this step's matmuls
                        # so they queue behind the cold-start critical
                        # stream: bias quarters at steps 3-6, the first two
                        # mg1 x chunks at steps 6-7; the rest of mg1's x
                        # rides two steps ahead of consumption inside mg1,
                        # and the next n-tile's W streams during each mg1.
                        if nt == 0 and mg == 0:
                            if 3 <= ko4 <= 6:
                                q = ko4 - 3
                                nc.sync.dma_start(
                                    bias_sb[:, q * 1024 : (q + 1) * 1024],
                                    bias_in[:, q * 1024 : (q + 1) * 1024],
                                )
                            if ko4 == 6:
                                xchunks[1][0] = load_xchunk(1, 0)
                            elif ko4 == 7:
                                xchunks[1][1] = load_xchunk(1, 1)
                        if nt == 0 and mg == 1 and ko4 + 2 < n_chunks:
                            xchunks[1][ko4 + 2] = load_xchunk(1, ko4 + 2)
                        if mg == m_groups - 1 and nt + 1 < NT:
                            wk_next[ko4] = load_wchunk(nt + 1, ko4)
                if nt + 1 < NT:
                    for c in range(n_chunks):
                        wk_cur[c] = wk_next[c]

    nc.compile()
    return nc


def prep_inputs(x, W, b, A, m_per_core: int = M_PER_CORE, n_cores: int = N_CORES):
    """Host-side shard + layout prep. Returns in_maps for run_bass_kernel_spmd."""
    x = np.asarray(x, dtype=np.float32)
    W = np.asarray(W, dtype=np.float32)
    b = np.asarray(b, dtype=np.float32)
    A = np.asarray(A, dtype=np.float32)

    r = A.shape[0]
    w_eff = W + np.tile(A, (OUT_F // r, IN_F // r))
    # wt[p, nt*KO + ko, j] = w_eff[nt*512 + j, ko*128 + p]
    wt = np.ascontiguousarray(
        w_eff.reshape(NT, NTILE, KO, P)
        .transpose(3, 0, 2, 1)
        .reshape(P, NT * KO, NTILE)
        .astype(NP_BF16)
    )
    bias = np.ascontiguousarray(np.broadcast_to(b, (P, OUT_F)))

    x_flat = x.reshape(TOKENS, IN_F)
    m_groups = m_per_core // MB
    in_maps = []
    for c in range(n_cores):
        shard = x_flat[c * m_per_core : (c + 1) * m_per_core]
        # xt[p, mg*KO + ko, m] = shard[mg*MB + m, ko*128 + p]
        xt = np.ascontiguousarray(
            shard.reshape(m_groups, MB, KO, P)
            .transpose(3, 0, 2, 1)
            .reshape(P, m_groups * KO, MB)
            .astype(NP_BF16)
        )
        in_maps.append({"xt": xt, "wt": wt, "bias": bias})
    return in_maps


def unshard(results, m_per_core: int = M_PER_CORE):
    shards = []
    for res in results:
        o = np.asarray(res["out"], dtype=np.float32)
        # [P, m_per_core//P, OUT_F]; token = strip*128 + p
        shards.append(o.transpose(1, 0, 2).reshape(m_per_core, OUT_F))
    return np.concatenate(shards, axis=0).reshape(B, S, OUT_F)


_NC_CACHE = {}


def run(x, W, b, A, trace=False, tmpdir=None, **spmd_kwargs):
    key = (M_PER_CORE, MB)
    if key not in _NC_CACHE:
        _NC_CACHE[key] = build_nc()
    nc = _NC_CACHE[key]
    in_maps = prep_inputs(x, W, b, A)
    br = run_bass_kernel_spmd(
        nc, in_maps, list(range(N_CORES)), trace=trace, tmpdir=tmpdir, **spmd_kwargs
    )
    return unshard(br.results), br


def kernel(x, W, b, A):
    last_err = None
    for attempt in range(3):
        try:
            out, _ = run(x, W, b, A)
            return out.astype(np.float32)
        except Exception as e:  # transient device flakes (e.g. NRT exec errors)
            last_err = e
            _NC_CACHE.clear()
            import time

            time.sleep(5)
    raise last_err


# revision 15
# speedup vs baseline: 1.2007x; 1.0016x over previous
"""MoRALinear fused kernel for 8x TRN2 NeuronCores — bf16 v5.

Math: reference computes
    y = x @ W.T + b + tile(lora_A(chunk_sum(x)))
Both the chunk-sum (x -> [B,S,r] by summing 4 chunks of 1024) and the
output tiling (repeat r->4096) are linear maps made of stacked identity
blocks, so the adapter folds into the base weight exactly:
    y = x @ (W + tile(A, (4,4))).T + b
The device kernel is then a single dense GEMM [16384,4096]x[4096,4096]
plus a bias, data-parallel over tokens across 8 cores (weights
replicated; no collectives).

Design (per core: M=2048 tokens, K=4096, N=4096):
  - Operands are bf16: the PE runs bf16 at the same 1 elem/cycle as
    float32r, but LDWEIGHTS gets FWL (hides fully under the matmul
    stream: measured 216 ns/MM = the N=512 issue floor) and HBM traffic
    halves. Accumulation stays fp32 in PSUM; rel err ~4e-3.
  - Loop order: n-tile outer, m-group inner. The whole 2048-token x
    shard is SBUF-resident (16 chunks, 128 KiB/part) and W streams
    exactly once (8 chunks per n-tile, reused by both m-groups) —
    minimum HBM traffic also keeps the package out of the sustained-
    power P0 downclock (a W-per-tile variant measured 2.0 GHz all run).
  - 8 PSUM banks hold the 8 m-strips of one (n-tile, m-group); bias is
    added during PSUM->SBUF eviction on the vector engine, emitting bf16
    (2x DVE rate, half the out-DMA bytes; host upcasts). The last
    k-chunk runs m-outer so the 8 evictions spread over ~7 us instead of
    bunching into a serial tail chain.
  - ~14 warm-up matmuls on a zeroed SBUF tile run during the initial
    DMA wait so the PE's HAM clock gate reaches 8/8 (2.4 GHz) before
    the first real matmul; the first tile streams per-k-strip with x
    leading W, and bias rides in quarters behind the cold-start stream.
"""

import numpy as np
import ml_dtypes

import concourse.bass as bass
import concourse.mybir as mybir
import concourse.tile as tile
from concourse import bacc
from concourse.bass_utils import run_bass_kernel_spmd

B, S, IN_F, OUT_F = 4, 4096, 4096, 4096
N_CORES = 8
TOKENS = B * S                  # 16384
M_PER_CORE = TOKENS // N_CORES  # 2048

P = 128
KO = IN_F // P                  # 32 k-strips
NTILE = 512
NT = OUT_F // NTILE             # 8 n-tiles
KO_CHUNK = 4                    # k-strips per chunk
MB = 1024                       # tokens per m-group (8 PSUM banks x 128)
N_WARM = 8                      # HAM warm-up matmuls

F32 = mybir.dt.float32
BF16 = mybir.dt.bfloat16
NP_BF16 = ml_dtypes.bfloat16


def build_nc(m_per_core: int = M_PER_CORE, mb: int = MB):
    assert m_per_core % mb == 0
    m_groups = m_per_core // mb          # 2
    n_chunks = KO // KO_CHUNK            # 8
    mt = mb // P                         # 8 m-strips per group
    nc = bacc.Bacc("TRN2", target_bir_lowering=False, debug=False)

    with tile.TileContext(nc) as tc:
        with tc.tile_pool(name="dram", bufs=1, space="DRAM") as dram:
            # x swizzled group-major: row mg*KO+ko holds k-strip ko of token
            # group mg, so each chunk load is one contiguous run/partition
            xt = dram.tile(
                [P, m_groups * KO, mb], BF16, kind="ExternalInput", name="xt",
                uniquify=False,
            )
            # W swizzled n-tile-major: row nt*KO+ko holds W_eff.T[k-strip ko,
            # n-slice nt] so each chunk load is one contiguous run/partition
            wt = dram.tile(
                [P, NT * KO, NTILE], BF16, kind="ExternalInput", name="wt",
                uniquify=False,
            )
            bias_in = dram.tile(
                [P, OUT_F], F32, kind="ExternalInput", name="bias", uniquify=False
            )
            out_d = dram.tile(
                [P, m_per_core // P, OUT_F], BF16, kind="ExternalOutput", name="out",
                uniquify=False,
            )

        with (
            tc.tile_pool(name="const", bufs=1) as const,
            tc.tile_pool(name="xpool", bufs=m_groups * n_chunks) as xpool,
            tc.tile_pool(name="wpool", bufs=n_chunks + 2) as wpool,
            tc.tile_pool(name="opool", bufs=8) as opool,
            tc.tile_pool(name="pspool", bufs=8, space="PSUM") as pspool,
        ):
            bias_sb = const.tile([P, OUT_F], F32, name="bias_sb")

            # HAM warm-up operands: zeroed by GpSimd (otherwise idle) so
            # the warm-up matmuls depend on nothing but the preamble
            warm_w = const.tile([P, P], BF16, name="warm_w")
            warm_x = const.tile([P, NTILE], BF16, name="warm_x")
            nc.gpsimd.memset(warm_w[:], 0.0)
            nc.gpsimd.memset(warm_x[:], 0.0)

            # x chunks stay resident for the whole kernel
            xchunks = [[None] * n_chunks for _ in range(m_groups)]
            wk_cur = [None] * n_chunks   # W chunks of the current n-tile
            wk_next = [None] * n_chunks  # prefetched chunks of the next n-tile

            def load_xchunk(mg, ko4):
                xc = xpool.tile([P, KO_CHUNK, mb], BF16, name="xc")
                row = mg * KO + ko4 * KO_CHUNK
                nc.sync.dma_start(xc[:], xt[:, row : row + KO_CHUNK, :])
                return xc

            def load_cold_chunks(ko4, halve_x):
                """Cold-start loads for (nt0, mg0): x and W interleaved per
                k-strip (x leading) so the ring arrival order matches the
                kj-inner consumption order; the first k-strip's x is split
                into token halves so the first matmuls start even earlier."""
                xc = xpool.tile([P, KO_CHUNK, mb], BF16, name="xc")
                wk = wpool.tile([P, KO_CHUNK, NTILE], BF16, name="wk")
                for kj in range(KO_CHUNK):
                    row = ko4 * KO_CHUNK + kj
                    if halve_x and kj == 0:
                        nc.sync.dma_start(
                            xc[:, 0:1, : mb // 2], xt[:, row : row + 1, : mb // 2]
                        )
                        nc.sync.dma_start(wk[:, 0:1, :], wt[:, row : row + 1, :])
                        nc.sync.dma_start(
                            xc[:, 0:1, mb // 2 :], xt[:, row : row + 1, mb // 2 :]
                        )
                    else:
                        nc.sync.dma_start(
                            xc[:, kj : kj + 1, :], xt[:, row : row + 1, :]
                        )
                        nc.sync.dma_start(
                            wk[:, kj : kj + 1, :], wt[:, row : row + 1, :]
                        )
                return xc, wk

            def load_wchunk(nt, ko4):
                wk = wpool.tile([P, KO_CHUNK, NTILE], BF16, name="wk")
                row = nt * KO + ko4 * KO_CHUNK
                nc.sync.dma_start(wk[:], wt[:, row : row + KO_CHUNK, :])
                return wk

            def do_mm(psums, xc, wk, kj, m, ko4, nt, mg):
                ko = ko4 * KO_CHUNK + kj
                nc.tensor.matmul(
                    psums[m][:],
                    lhsT=xc[:, kj : kj + 1, m * P : (m + 1) * P],
                    rhs=wk[:, kj : kj + 1, :],
                    start=(ko == 0),
                    stop=(ko == KO - 1),
                )
                if ko == KO - 1:
                    ns = slice(nt * NTILE, (nt + 1) * NTILE)
                    ot = opool.tile([P, NTILE], BF16, name="ot")
                    nc.vector.tensor_add(
                        out=ot[:], in0=psums[m][:], in1=bias_sb[:, ns]
                    )
                    nc.sync.dma_start(out_d[:, mg * mt + m, ns], ot[:])

            for nt in range(NT):
                for mg in range(m_groups):
                    psums = [
                        pspool.tile([P, NTILE], F32, name="ps") for _ in range(mt)
                    ]
                    if nt == 0 and mg == 0:
                        # HAM warm-up into psums[0]: the first real matmul's
                        # start=True clears the bank, so nothing escapes
                        for _ in range(N_WARM):
                            nc.tensor.matmul(
                                psums[0][:], lhsT=warm_w[:], rhs=warm_x[:],
                                start=True, stop=True,
                            )
                    for ko4 in range(n_chunks):
                        if nt == 0 and mg == 0:
                            xchunks[0][ko4], wk_cur[ko4] = load_cold_chunks(
                                ko4, halve_x=(ko4 == 0)
                            )
                        wk = wk_cur[ko4]
                        xc = xchunks[mg][ko4]
                        if ko4 == n_chunks - 1:
                            # m-outer: each strip finishes its k-accumulation
                            # 4 MMs apart, spreading evictions across ~7us
                            for m in range(mt):
                                for kj in range(KO_CHUNK):
                                    do_mm(psums, xc, wk, kj, m, ko4, nt, mg)
                        else:
                            for kj in range(KO_CHUNK):
                                for m in range(mt):
                                    do_mm(psums, xc, wk, kj, m, ko4, nt, mg)
                        # trailing loads, emitted after this step's matmuls
                        # so they queue behind the cold-start critical
                        # stream: bias quarters at steps 3-6, the first two
                        # mg1 x chunks at steps 6-7; the rest of mg1's x
                        # rides two steps ahead of consumption inside mg1,
                        # and the next n-tile's W streams during each mg1.
                        if nt == 0 and mg == 0:
                            if 3 <= ko4 <= 6:
                                q = ko4 - 3
                                nc.sync.dma_start(
                                    bias_sb[:, q * 1024 : (q + 1) * 1024],
                                    bias_in[:, q * 1024 : (q + 1) * 1024],
                                )
                            if ko4 == 6:
                                xchunks[1][0] = load_xchunk(1, 0)
                            elif ko4 == 7:
                                xchunks[1][1] = load_xchunk(1, 1)
                        if nt == 0 and mg == 1 and ko4 + 2 < n_chunks:
                            xchunks[1][ko4 + 2] = load_xchunk(1, ko4 + 2)
                        if mg == m_groups - 1 and nt + 1 < NT:
                            wk_next[ko4] = load_wchunk(nt + 1, ko4)
                if nt + 1 < NT:
                    for c in range(n_chunks):
                        wk_cur[c] = wk_next[c]

    nc.compile()
    return nc


def prep_inputs(x, W, b, A, m_per_core: int = M_PER_CORE, n_cores: int = N_CORES):
    """Host-side shard + layout prep. Returns in_maps for run_bass_kernel_spmd."""
    x = np.asarray(x, dtype=np.float32)
    W = np.asarray(W, dtype=np.float32)
    b = np.asarray(b, dtype=np.float32)
    A = np.asarray(A, dtype=np.float32)

    r = A.shape[0]
    w_eff = W + np.tile(A, (OUT_F // r, IN_F // r))
    # wt[p, nt*KO + ko, j] = w_eff[nt*512 + j, ko*128 + p]
    wt = np.ascontiguousarray(
        w_eff.reshape(NT, NTILE, KO, P)
        .transpose(3, 0, 2, 1)
        .reshape(P, NT * KO, NTILE)
        .astype(NP_BF16)
    )
    bias = np.ascontiguousarray(np.broadcast_to(b, (P, OUT_F)))

    x_flat = x.reshape(TOKENS, IN_F)
    m_groups = m_per_core // MB
    in_maps = []
    for c in range(n_cores):
        shard = x_flat[c * m_per_core : (c + 1) * m_per_core]
        # xt[p, mg*KO + ko, m] = shard[mg*MB + m, ko*128 + p]
        xt = np.ascontiguousarray(
            shard.reshape(m_groups, MB, KO, P)
            .transpose(3, 0, 2, 1)
            .reshape(P, m_groups * KO, MB)
            .astype(NP_BF16)
        )
        in_maps.append({"xt": xt, "wt": wt, "bias": bias})
    return in_maps


def unshard(results, m_per_core: int = M_PER_CORE):
    shards = []
    for res in results:
        o = np.asarray(res["out"], dtype=np.float32)
        # [P, m_per_core//P, OUT_F]; token = strip*128 + p
        shards.append(o.transpose(1, 0, 2).reshape(m_per_core, OUT_F))
    return np.concatenate(shards, axis=0).reshape(B, S, OUT_F)


_NC_CACHE = {}


def run(x, W, b, A, trace=False, tmpdir=None, **spmd_kwargs):
    key = (M_PER_CORE, MB)
    if key not in _NC_CACHE:
        _NC_CACHE[key] = build_nc()
    nc = _NC_CACHE[key]
    in_maps = prep_inputs(x, W, b, A)
    br = run_bass_kernel_spmd(
        nc, in_maps, list(range(N_CORES)), trace=trace, tmpdir=tmpdir, **spmd_kwargs
    )
    return unshard(br.results), br


def kernel(x, W, b, A):
    last_err = None
    for attempt in range(3):
        try:
            out, _ = run(x, W, b, A)
            return out.astype(np.float32)
        except Exception as e:  # transient device flakes (e.g. NRT exec errors)
            last_err = e
            _NC_CACHE.clear()
            import time

            time.sleep(5)
    raise last_err
